# revision 74
# baseline (speedup 1.0000x reference)
"""Trainium2 Bass/Tile kernel: supervised contrastive loss (N=8192, D=256).

Reference math (jax): r = x / max(||x||, 1e-12); sim = r @ r.T;
  neg_ij = (label_i != label_j); den_i = sum_j exp(sim_ij * neg_ij / 0.1) + 1
  loss = mean_i log(den_i + 1e-8)
Since exp(sim_ij * neg_ij / T) == 1 for every same-label pair (incl. the
diagonal), den_i = sum_{j: l_j != l_i} exp(sim_ij/T) + count_same_i + 1 with
count_same_i = #{j: l_j == l_i} (including j == i).

Device strategy (8 NeuronCores, SPMD, row-parallel): each core computes its
1024-row slice of exp(sim/T) against all 8192 columns and reduces locally;
the host sums the 8 per-core partial log-den sums ("all-reduce the mean").
Host prep is layout/label-only: x^T cast to bf16 (own rows) and fp8
(columns), labels as one-hot channel matrices, per-row same-label counts
from the label histogram.

The 8M-element exp() is the hard floor (ACT: 1 elem/cycle/partition at
1.2 GHz => 54.6us if ACT did everything), so the design splits the exp
across TWO psum-capable engines and keeps everything else off their path:

  * TRANSPOSED main loop: psum[j-tile, own-i] = stationary fp8 RF column
    block x moving own rows (fp8 DoubleRow, K=256 in one pass).  The
    row-sum over j becomes a contraction over the PARTITION axis: 1-wide
    PE matmuls of the exp output against a ones vector, accumulated across
    all 64 j-tiles in one psum bank.  No ACT accumulator reads, no vector
    reductions, and exp results never return to HBM.
  * The exp is SPLIT by tile between ACT (table exp, bf16 out) and DVE
    (Schraudolph fast exp: one tensor_scalar computing int32(x*A + B)
    whose bitcast is 2^(x*log2(e)+...) to ~2% elementwise, mean-zero
    tuned; the per-row sum over ~8k terms averages the error to ~1e-4).
    Pool (GPSIMD) cannot touch PSUM, so it instead computes the squares
    for the norms, SBUF->SBUF.
  * Normalization never touches operand-shaped data: the raw fp8 columns
    go straight to the PE, and 1/norm is folded into the exp as a
    PER-PARTITION scale vector (ACT scale operand / tensor_scalar scalar
    AP).  Norms are computed packed: Pool squares -> per-128-column
    sums-of-squares via 1-wide matmuls (squares stationary) -> ln+exp on
    just [128, 8] tiles (one shared ACT table with the main exp).
    Software-pipelined three windows deep (squares two groups ahead,
    ln/exp+scales one group ahead) so no engine queue ever head-of-line
    blocks on the chain.
  * The same-label mask is folded into the matmul: -80 * one-hot label
    channels as a second fp8 DoubleRow pass; exp((sim - 80*same) * 10/|x|)
    vanishes for same-label pairs and the diagonal. count_same is restored
    exactly from the host histogram.
  * Own rows ARE normalized as an fp8 operand (moving side cannot use the
    scale trick): packed inv -> PE-transpose -> selector-matmul broadcast
    -> 4 DVE multiplies, all during startup.
  * Finale on-device: den = colsum + count + 1 -> ln -> per-core partial
    sum via fp32 matmul with ones -> 4-byte DMA out.
"""

import numpy as np
import ml_dtypes

N = 8192
D = 256
NCORES = 8
OWN = N // NCORES          # 1024 rows per core
MT = OWN // 128            # 8 row tiles per core
NT = N // 128              # 64 column tiles
ISCALE = 10.0              # 1 / temperature
CHUNK = 512                # matmul free-dim tile
GRP = 1024                 # column group width for norm staging
NG = N // GRP              # 8 column groups
GT = GRP // 128            # 8 column tiles per group
LAG = 4                    # j-tiles between exp and its rowsum matmuls
PLAN_G = [
    ["A", "D", "A", "D", "A", "A", "D", "A"],
    ["D", "A", "D", "A", "D", "A", "D", "A"],
    ["A", "D", "A", "D", "A", "A", "D", "A"],
    ["D", "A", "D", "A", "D", "A", "D", "A"],
    ["A", "D", "A", "D", "A", "A", "D", "A"],
    ["D", "A", "D", "A", "D", "A", "D", "A"],
    ["A", "D", "A", "D", "A", "A", "D", "A"],
    ["D", "A", "D", "A", "D", "A", "D", "A"],
]

_CACHE = {}


def _build():
    import concourse.bass as bass
    import concourse.tile as tile
    import concourse.bacc as bacc_mod
    from concourse import bacc, mybir
    from contextlib import ExitStack

    f32 = mybir.dt.float32
    bf16 = mybir.dt.bfloat16
    f8 = mybir.dt.float8e4
    Act = mybir.ActivationFunctionType
    AX = mybir.AxisListType.X
    AP = bass.AP
    DR = mybir.MatmulPerfMode.DoubleRow
    Alu = mybir.AluOpType

    # Schraudolph fast-exp constants: exp(10*x) ~ bitcast(int32(x*SA + SB))
    # with SB's offset tuned for zero mean error over uniform mantissa frac
    _ln2 = float(np.log(2.0))
    _i0 = 1.0 / (2.0 * _ln2)
    _i1 = (1.0 - (1.0 + _ln2) * float(np.exp(-_ln2))) / (_ln2 ** 2)
    _cp = 1.0 - (1.0 - _i1) / _i0
    SA = float(ISCALE * (1 << 23) / _ln2)
    SB = float((127.0 - _cp) * (1 << 23))

    # Force Exp and Ln to resolve to the one table set that holds both, so
    # interleaved ln/exp never reloads ACT tables.
    orig_gat = bacc_mod.get_activation_tables

    def gat_shared(arch):
        tabs = orig_gat(arch)
        for name, fns in tabs.items():
            if name != "natural_log_exp_and_others":
                fns.discard(Act.Exp)
                fns.discard(Act.Ln)
        return tabs

    bacc_mod.get_activation_tables = gat_shared
    try:
        nc = bacc.Bacc("TRN2", target_bir_lowering=False, debug=False,
                       num_devices=NCORES)

        xt8_d = nc.dram_tensor("xt8", [D, N], f8, kind="ExternalInput")
        xto_d = nc.dram_tensor("xto", [D, OWN], bf16, kind="ExternalInput")
        ohj_d = nc.dram_tensor("ohj", [256, N], f8, kind="ExternalInput")
        ohm_d = nc.dram_tensor("ohm", [256, OWN], f8, kind="ExternalInput")
        cnt_d = nc.dram_tensor("cnt", [128, MT], f32, kind="ExternalInput")
        out_d = nc.dram_tensor("out", [1, 1], f32, kind="ExternalOutput")

        cb_d = nc.inline_tensor(
            np.concatenate([np.ones((128, 1)), np.eye(128)],
                           axis=1).astype(ml_dtypes.bfloat16), "cb_c")
        cf_d = nc.inline_tensor(
            np.concatenate([np.ones((128, 1)),
                            np.full((128, 1), 1e-12)],
                           axis=1).astype(np.float32), "cf_c")
        sels_d = nc.inline_tensor(
            np.kron(np.eye(16), np.ones((1, 128))).astype(
                ml_dtypes.bfloat16), "sels_c")

        with tile.TileContext(nc) as tc:
            with ExitStack() as top:
                persist = top.enter_context(
                    tc.tile_pool(name="persist", bufs=1))
                work = top.enter_context(tc.tile_pool(name="work", bufs=3))
                expool = top.enter_context(
                    tc.tile_pool(name="expool", bufs=LAG + 3))
                psum = top.enter_context(
                    tc.tile_pool(name="psum", bufs=3, space="PSUM"))
                npsum = top.enter_context(
                    tc.tile_pool(name="npsum", bufs=1, space="PSUM"))
                dpsum = top.enter_context(
                    tc.tile_pool(name="dpsum", bufs=1, space="PSUM"))

                RF = persist.tile([128, 2, N], f8)      # normalized x^T fp8
                RFO = persist.tile([128, 2, OWN], f8)   # own rows fp8
                OHJ = persist.tile([128, 2, N], f8)     # one-hot (slab1=0)
                OHM = persist.tile([128, 2, OWN], f8)   # -80*one-hot own
                XO = persist.tile([128, 2, OWN], bf16)
                SO = persist.tile([128, 2, OWN], bf16)
                CNT = persist.tile([128, MT], f32)
                DEN = persist.tile([128, MT], f32)
                T0 = persist.tile([128, MT], f32)
                LV = persist.tile([128, MT], f32)
                LS = persist.tile([128, 1], f32)
                CB = persist.tile([128, 129], bf16)
                CF = persist.tile([128, 2], f32)
                sels_sb = persist.tile([16, 2048], bf16)
                outsb = persist.tile([1, 1], f32)
                onesb_sb = CB[:, 0:1]
                ident_sb = CB[:, 1:129]
                onesf_sb = CF[:, 0:1]
                beps_sb = CF[:, 1:2]

                def sumsq_lnexp(sqa, sqb, ntiles, invp):
                    """Packed norms: per-128-col-tile sum of squares via
                    1-wide matmuls (squares stationary, ones moving), then
                    inv = exp(-0.5*ln(s)) on [128, ntiles] only."""
                    ps = npsum.tile([128, 16], f32, tag="ns")
                    for t in range(ntiles):
                        sl = slice(t * 128, (t + 1) * 128)
                        nc.tensor.matmul(ps[:, t:t + 1], sqa[:, sl],
                                         onesb_sb, start=True, stop=False)
                        nc.tensor.matmul(ps[:, t:t + 1], sqb[:, sl],
                                         onesb_sb, start=False, stop=True)
                    lnv = work.tile([128, 16], f32, tag="lnv")
                    nc.scalar.activation(lnv[:, 0:ntiles], ps[:, 0:ntiles],
                                         Act.Ln)
                    nc.scalar.activation(invp, lnv[:, 0:ntiles], Act.Exp,
                                         scale=-0.5)

                def unpack_inv(invp, ntiles):
                    """Packed inv [128, ntiles] -> row layout [ntiles,
                    128] via PE transpose, staged to SBUF.  Broadcasting to
                    operand shape happens per 512-chunk in bcast_chunk."""
                    trp = npsum.tile([16, 128], bf16, tag="ns")
                    nc.tensor.transpose(trp[0:ntiles, :], invp,
                                        ident_sb)
                    trs = work.tile([16, 128], bf16, tag="trs")
                    nc.vector.tensor_copy(trs[0:ntiles, :],
                                          trp[0:ntiles, :])
                    return trs

                def load_group(g):
                    c0 = g * GRP
                    nc.sync.dma_start(
                        RF[:, :, c0:c0 + GRP],
                        AP(xt8_d, c0, [[N, 128], [128 * N, 2], [1, GRP]]))

                def load_oh(g):
                    gs = slice(g * GRP, (g + 1) * GRP)
                    nc.sync.dma_start(
                        OHJ[:, :, gs],
                        AP(ohj_d, g * GRP, [[N, 128], [128 * N, 2],
                                            [1, GRP]]))

                # ---- bulk loads first: the SP DMA queue must never
                # stall behind a dependency-gated transfer; each dma has a
                # ~625ns fixed cost so order = need order ----

                nc.sync.dma_start(
                    XO, AP(xto_d, 0, [[OWN, 128], [128 * OWN, 2],
                                      [1, OWN]]))
                nc.sync.dma_start(CB, cb_d[:])
                dumt = work.tile([128, 1], f32, tag="dum")
                nc.scalar.activation(dumt, onesb_sb, Act.Exp)
                nc.vector.tensor_mul(SO, XO, XO)
                invpo = work.tile([128, 16], bf16, tag="invpo")
                sumsq_lnexp(SO[:, 0, :], SO[:, 1, :], MT, invpo[:, 0:MT])

                # ---- global norm chain, per group ----

                def sq_stage(g, eng):
                    """Squares of group g's fp8 columns (SBUF->SBUF;
                    Pool-legal).  Emitted ~2 group-windows before use so
                    the slow Pool multiply never blocks a queue."""
                    gs = slice(g * GRP, (g + 1) * GRP)
                    sq2 = work.tile([128, 2, GRP], bf16, tag="sq2")
                    eng.tensor_mul(sq2, RF[:, :, gs], RF[:, :, gs])
                    return sq2

                def fin_stage(g, sq2):
                    """sumsq matmuls + packed ln/exp + ACT scale vectors
                    SCA (table exp, 10*inv) / SCP (Schraudolph, SA*inv).
                    Emitted one group-window before use."""
                    invp = work.tile([128, 16], f32, tag="invp")
                    sumsq_lnexp(sq2[:, 0, :], sq2[:, 1, :], GT,
                                invp[:, 0:GT])
                    sca = work.tile([128, GT], f32, tag="sca")
                    scp = work.tile([128, GT], f32, tag="scp")
                    nc.vector.tensor_scalar_mul(sca, invp[:, 0:GT], ISCALE)
                    nc.vector.tensor_scalar_mul(scp, invp[:, 0:GT], SA)
                    return sca, scp

                nc.sync.dma_start(sels_sb, sels_d[:])
                nc.sync.dma_start(CF, cf_d[:])
                load_group(0)
                load_group(1)
                load_group(2)
                load_group(3)
                load_oh(0)
                nc.sync.dma_start(
                    OHM, AP(ohm_d, 0, [[OWN, 128], [128 * OWN, 2],
                                       [1, OWN]]))
                load_oh(1)
                # own-row unpack + mults (gates the first main matmul);
                # bc tiles borrow the still-idle mm tag so the two chunk
                # chains overlap instead of ping-ponging on the ns tag
                trso = unpack_inv(invpo[:, 0:MT], MT)
                for c in range(OWN // CHUNK):
                    cs = slice(c * CHUNK, (c + 1) * CHUNK)
                    bc = psum.tile([128, OWN], f32, tag="mm")
                    for i in range(4):
                        t = c * 4 + i
                        nc.tensor.matmul(
                            bc[:, i * 128:(i + 1) * 128],
                            sels_sb[0:MT, t * 128:(t + 1) * 128],
                            trso[0:MT, :], start=True, stop=True)
                    nc.vector.tensor_mul(RFO[:, 0, cs], XO[:, 0, cs],
                                         bc[:, 0:512])
                    nc.vector.tensor_mul(RFO[:, 1, cs], XO[:, 1, cs],
                                         bc[:, 0:512])
                sq_g = {}
                sq_g[0] = sq_stage(0, nc.gpsimd)
                sq_g[1] = sq_stage(1, nc.vector)
                sc0 = fin_stage(0, sq_g.pop(0))

                # ---- main loop: 64 j-tiles, transposed orientation ----
                DENPS = dpsum.tile([128, MT], f32, tag="den")
                pending = []

                def flush_sums(limit):
                    # one psum accumulation group spans the whole DENPS
                    # bank: exactly one start and one stop
                    while len(pending) > limit:
                        t, ext, isf32 = pending.pop(0)
                        for s in range(MT):
                            sl = ext[:, s * 128:(s + 1) * 128]
                            st = (t == 0 and s == 0)
                            sp = (t == NT - 1 and s == MT - 1)
                            if isf32:
                                nc.tensor.matmul(
                                    DENPS[:, s:s + 1], sl.bitcast(f32),
                                    onesf_sb, start=st, stop=sp)
                            else:
                                nc.tensor.matmul(
                                    DENPS[:, s:s + 1], sl, onesb_sb,
                                    start=st, stop=sp)

                def main_tile(t, dve, sc):
                    # dve: "A" = ACT table exp, "D"/"P" = Schraudolph
                    sca, scp = sc
                    tl = t % GT
                    tb = slice(t * 128, (t + 1) * 128)
                    ps = psum.tile([128, OWN], f32, tag="mm")
                    for c in range(OWN // CHUNK):
                        cs = slice(c * CHUNK, (c + 1) * CHUNK)
                        nc.tensor.matmul(ps[:, cs], RF[:, :, tb],
                                         RFO[:, :, cs],
                                         start=True, stop=False,
                                         perf_mode=DR)
                    for c in range(OWN // CHUNK):
                        cs = slice(c * CHUNK, (c + 1) * CHUNK)
                        nc.tensor.matmul(ps[:, cs], OHJ[:, :, tb],
                                         OHM[:, :, cs],
                                         start=False, stop=True,
                                         perf_mode=DR)
                    if dve == "D":
                        # Schraudolph fast exp on DVE (Pool cannot read
                        # PSUM, so the split is ACT/DVE only)
                        ext = expool.tile([128, OWN], mybir.dt.int32,
                                          tag="exi")
                        nc.vector.tensor_scalar(
                            out=ext, in0=ps, scalar1=scp[:, tl:tl + 1],
                            scalar2=SB, op0=Alu.mult, op1=Alu.add)
                        pending.append((t, ext, True))
                    else:
                        ext = expool.tile([128, OWN], bf16, tag="ext")
                        nc.scalar.activation(ext, ps, Act.Exp,
                                             scale=sca[:, tl:tl + 1])
                        pending.append((t, ext, False))
                    flush_sums(LAG)


                sc = sc0
                sc_next = None
                for g in range(NG):
                    plan = PLAN_G[g]
                    for m in range(GT):
                        main_tile(g * GT + m, plan[m], sc)
                        if m == 1:
                            if g + 4 < NG:
                                load_group(g + 4)
                            if g + 2 < NG:
                                load_oh(g + 2)
                        if m == 6 and g == 0:
                            # CNT is only read in the finale; queue it
                            # after all startup-critical transfers
                            nc.sync.dma_start(CNT, cnt_d[:])
                        if m == 3 and g + 2 < NG:
                            sq_g[g + 2] = sq_stage(g + 2, nc.gpsimd)
                        if m == 6 and g + 1 < NG:
                            sc_next = fin_stage(g + 1, sq_g.pop(g + 1))
                    sc = sc_next
                flush_sums(0)

                # ---- finale: den = colsum + count + 1 -> log ----
                # stt reads DENPS straight from psum; ln fuses the row
                # reduce via its accumulator
                nc.vector.scalar_tensor_tensor(
                    out=DEN, in0=DENPS, scalar=1.0, in1=CNT,
                    op0=mybir.AluOpType.add, op1=mybir.AluOpType.add)
                nc.scalar.activation(LV, DEN, Act.Ln, accum_out=LS)
                psf = psum.tile([1, 1], f32, tag="mm")
                nc.tensor.matmul(psf, LS, onesf_sb, start=True, stop=True)
                nc.vector.tensor_copy(outsb, psf)
                nc.sync.dma_start(out_d[:], outsb)

        nc.compile()
    finally:
        bacc_mod.get_activation_tables = orig_gat
    return nc


def _get_nc():
    if "nc" not in _CACHE:
        _CACHE["nc"] = _build()
    return _CACHE["nc"]


def _make_in_maps(representations, pseudo_labels):
    x = np.asarray(representations, dtype=np.float32)
    labels = np.asarray(pseudo_labels).astype(np.int32).reshape(N)
    xt = np.ascontiguousarray(x.T).astype(ml_dtypes.bfloat16)   # [256, N]
    xt8 = xt.astype(ml_dtypes.float8_e4m3)                      # [256, N]

    oh = (labels[None, :] == np.arange(128, dtype=np.int32)[:, None])
    # one-hot channels with a zeroed twin slab (fp8 DoubleRow operand)
    ohj = np.zeros((256, N), dtype=ml_dtypes.float8_e4m3)
    ohj[0:128] = oh
    counts = np.bincount(labels, minlength=128).astype(np.float32)
    cnt_row = counts[labels]                                    # [N]

    in_maps = []
    for c in range(NCORES):
        lo, hi = c * OWN, (c + 1) * OWN
        ohm = np.zeros((256, OWN), dtype=ml_dtypes.float8_e4m3)
        ohm[0:128] = -80.0 * oh[:, lo:hi]
        cnt = np.ascontiguousarray(
            cnt_row[lo:hi].reshape(MT, 128).T).astype(np.float32)
        in_maps.append({
            "xt8": xt8,
            "xto": np.ascontiguousarray(xt[:, lo:hi]),
            "ohj": ohj,
            "ohm": ohm,
            "cnt": cnt,
        })
    return in_maps


def kernel(representations, pseudo_labels):
    from concourse.bass_utils import run_bass_kernel_spmd

    nc = _get_nc()
    in_maps = _make_in_maps(representations, pseudo_labels)
    res = run_bass_kernel_spmd(nc, in_maps, list(range(NCORES)))
    total = np.sum([np.float64(res.results[c]["out"][0, 0])
                    for c in range(NCORES)])
    return np.float32(total / N)


# revision 85
# speedup vs baseline: 1.0020x; 1.0020x over previous
"""Trainium2 Bass/Tile kernel: supervised contrastive loss (N=8192, D=256).

Reference math (jax): r = x / max(||x||, 1e-12); sim = r @ r.T;
  neg_ij = (label_i != label_j); den_i = sum_j exp(sim_ij * neg_ij / 0.1) + 1
  loss = mean_i log(den_i + 1e-8)
Since exp(sim_ij * neg_ij / T) == 1 for every same-label pair (incl. the
diagonal), den_i = sum_{j: l_j != l_i} exp(sim_ij/T) + count_same_i + 1 with
count_same_i = #{j: l_j == l_i} (including j == i).

Device strategy (8 NeuronCores, SPMD, row-parallel): each core computes its
1024-row slice of exp(sim/T) against all 8192 columns and reduces locally;
the host sums the 8 per-core partial log-den sums ("all-reduce the mean").
Host prep is layout/label-only: x^T cast to bf16 (own rows) and fp8
(columns), labels as one-hot channel matrices, per-row same-label counts
from the label histogram.

The 8M-element exp() is the hard floor (ACT: 1 elem/cycle/partition at
1.2 GHz => 54.6us if ACT did everything), so the design splits the exp
across TWO psum-capable engines and keeps everything else off their path:

  * TRANSPOSED main loop: psum[j-tile, own-i] = stationary fp8 RF column
    block x moving own rows (fp8 DoubleRow, K=256 in one pass).  The
    row-sum over j becomes a contraction over the PARTITION axis: 1-wide
    PE matmuls of the exp output against a ones vector, accumulated across
    all 64 j-tiles in one psum bank.  No ACT accumulator reads, no vector
    reductions, and exp results never return to HBM.
  * The exp is SPLIT by tile between ACT (table exp, bf16 out) and DVE
    (Schraudolph fast exp: one tensor_scalar computing int32(x*A + B)
    whose bitcast is 2^(x*log2(e)+...) to ~2% elementwise, mean-zero
    tuned; the per-row sum over ~8k terms averages the error to ~1e-4).
    Pool (GPSIMD) cannot touch PSUM, so it instead computes the squares
    for the norms, SBUF->SBUF.
  * Normalization never touches operand-shaped data: the raw fp8 columns
    go straight to the PE, and 1/norm is folded into the exp as a
    PER-PARTITION scale vector (ACT scale operand / tensor_scalar scalar
    AP).  Norms are computed packed: Pool squares -> per-128-column
    sums-of-squares via 1-wide matmuls (squares stationary) -> ln+exp on
    just [128, 8] tiles (one shared ACT table with the main exp).
    Software-pipelined three windows deep (squares two groups ahead,
    ln/exp+scales one group ahead) so no engine queue ever head-of-line
    blocks on the chain.
  * The same-label mask is folded into the matmul: -80 * one-hot label
    channels as a second fp8 DoubleRow pass; exp((sim - 80*same) * 10/|x|)
    vanishes for same-label pairs and the diagonal. count_same is restored
    exactly from the host histogram.
  * Own rows ARE normalized as an fp8 operand (moving side cannot use the
    scale trick): packed inv -> PE-transpose -> selector-matmul broadcast
    -> 4 DVE multiplies, all during startup.
  * Finale on-device: den = colsum + count + 1 -> ln -> per-core partial
    sum via fp32 matmul with ones -> 4-byte DMA out.
"""

import numpy as np
import ml_dtypes

N = 8192
D = 256
NCORES = 8
OWN = N // NCORES          # 1024 rows per core
MT = OWN // 128            # 8 row tiles per core
NT = N // 128              # 64 column tiles
ISCALE = 10.0              # 1 / temperature
CHUNK = 512                # matmul free-dim tile
GRP = 1024                 # column group width for norm staging
NG = N // GRP              # 8 column groups
GT = GRP // 128            # 8 column tiles per group
LAG = 4                    # j-tiles between exp and its rowsum matmuls
PLAN_G = [
    ["A", "D", "A", "D", "A", "A", "D", "A"],
    ["D", "A", "D", "A", "D", "A", "D", "A"],
    ["A", "D", "A", "D", "A", "A", "D", "A"],
    ["D", "A", "D", "A", "D", "A", "D", "A"],
    ["A", "D", "A", "D", "A", "A", "D", "A"],
    ["D", "A", "D", "A", "D", "A", "D", "A"],
    ["A", "D", "A", "D", "A", "A", "D", "A"],
    ["D", "A", "D", "A", "D", "A", "D", "A"],
]

_CACHE = {}


def _build():
    import concourse.bass as bass
    import concourse.tile as tile
    import concourse.bacc as bacc_mod
    from concourse import bacc, mybir
    from contextlib import ExitStack

    f32 = mybir.dt.float32
    bf16 = mybir.dt.bfloat16
    f8 = mybir.dt.float8e4
    Act = mybir.ActivationFunctionType
    AX = mybir.AxisListType.X
    AP = bass.AP
    DR = mybir.MatmulPerfMode.DoubleRow
    Alu = mybir.AluOpType

    # Schraudolph fast-exp constants: exp(10*x) ~ bitcast(int32(x*SA + SB))
    # with SB's offset tuned for zero mean error over uniform mantissa frac
    _ln2 = float(np.log(2.0))
    _i0 = 1.0 / (2.0 * _ln2)
    _i1 = (1.0 - (1.0 + _ln2) * float(np.exp(-_ln2))) / (_ln2 ** 2)
    _cp = 1.0 - (1.0 - _i1) / _i0
    SA = float(ISCALE * (1 << 23) / _ln2)
    SB = float((127.0 - _cp) * (1 << 23))

    # Force Exp and Ln to resolve to the one table set that holds both, so
    # interleaved ln/exp never reloads ACT tables.
    orig_gat = bacc_mod.get_activation_tables

    def gat_shared(arch):
        tabs = orig_gat(arch)
        for name, fns in tabs.items():
            if name != "natural_log_exp_and_others":
                fns.discard(Act.Exp)
                fns.discard(Act.Ln)
        return tabs

    bacc_mod.get_activation_tables = gat_shared
    try:
        nc = bacc.Bacc("TRN2", target_bir_lowering=False, debug=False,
                       num_devices=NCORES)

        xt8_d = nc.dram_tensor("xt8", [D, N], f8, kind="ExternalInput")
        xto_d = nc.dram_tensor("xto", [D, OWN], bf16, kind="ExternalInput")
        ohj_d = nc.dram_tensor("ohj", [256, N], f8, kind="ExternalInput")
        ohm_d = nc.dram_tensor("ohm", [256, OWN], f8, kind="ExternalInput")
        cnt_d = nc.dram_tensor("cnt", [128, MT], f32, kind="ExternalInput")
        out_d = nc.dram_tensor("out", [1, 1], f32, kind="ExternalOutput")

        cb_d = nc.inline_tensor(
            np.concatenate([np.ones((128, 1)), np.eye(128)],
                           axis=1).astype(ml_dtypes.bfloat16), "cb_c")
        cf_d = nc.inline_tensor(
            np.concatenate([np.ones((128, 1)),
                            np.full((128, 1), 1e-12)],
                           axis=1).astype(np.float32), "cf_c")
        sels_d = nc.inline_tensor(
            np.kron(np.eye(16), np.ones((1, 128))).astype(
                ml_dtypes.bfloat16), "sels_c")

        with tile.TileContext(nc) as tc:
            with ExitStack() as top:
                persist = top.enter_context(
                    tc.tile_pool(name="persist", bufs=1))
                work = top.enter_context(tc.tile_pool(name="work", bufs=5))
                expool = top.enter_context(
                    tc.tile_pool(name="expool", bufs=LAG + 8))
                psum = top.enter_context(
                    tc.tile_pool(name="psum", bufs=3, space="PSUM"))
                npsum = top.enter_context(
                    tc.tile_pool(name="npsum", bufs=1, space="PSUM"))
                dpsum = top.enter_context(
                    tc.tile_pool(name="dpsum", bufs=1, space="PSUM"))

                RF = persist.tile([128, 2, N], f8)      # normalized x^T fp8
                RFO = persist.tile([128, 2, OWN], f8)   # own rows fp8
                OHJ = persist.tile([128, 2, N], f8)     # one-hot (slab1=0)
                OHM = persist.tile([128, 2, OWN], f8)   # -80*one-hot own
                XO = persist.tile([128, 2, OWN], bf16)
                SO = persist.tile([128, 2, OWN], bf16)
                CNT = persist.tile([128, MT], f32)
                DEN = persist.tile([128, MT], f32)
                T0 = persist.tile([128, MT], f32)
                LV = persist.tile([128, MT], f32)
                LS = persist.tile([128, 1], f32)
                CB = persist.tile([128, 129], bf16)
                CF = persist.tile([128, 2], f32)
                sels_sb = persist.tile([16, 2048], bf16)
                outsb = persist.tile([1, 1], f32)
                onesb_sb = CB[:, 0:1]
                ident_sb = CB[:, 1:129]
                onesf_sb = CF[:, 0:1]
                beps_sb = CF[:, 1:2]

                def sumsq_lnexp(sqa, sqb, ntiles, invp):
                    """Packed norms: per-128-col-tile sum of squares via
                    1-wide matmuls (squares stationary, ones moving), then
                    inv = exp(-0.5*ln(s)) on [128, ntiles] only."""
                    ps = npsum.tile([128, 16], f32, tag="ns")
                    for t in range(ntiles):
                        sl = slice(t * 128, (t + 1) * 128)
                        nc.tensor.matmul(ps[:, t:t + 1], sqa[:, sl],
                                         onesb_sb, start=True, stop=False)
                        nc.tensor.matmul(ps[:, t:t + 1], sqb[:, sl],
                                         onesb_sb, start=False, stop=True)
                    lnv = work.tile([128, 16], f32, tag="lnv")
                    nc.scalar.activation(lnv[:, 0:ntiles], ps[:, 0:ntiles],
                                         Act.Ln)
                    nc.scalar.activation(invp, lnv[:, 0:ntiles], Act.Exp,
                                         scale=-0.5)

                def unpack_inv(invp, ntiles):
                    """Packed inv [128, ntiles] -> row layout [ntiles,
                    128] via PE transpose, staged to SBUF.  Broadcasting to
                    operand shape happens per 512-chunk in bcast_chunk."""
                    trp = npsum.tile([16, 128], bf16, tag="ns")
                    nc.tensor.transpose(trp[0:ntiles, :], invp,
                                        ident_sb)
                    trs = work.tile([16, 128], bf16, tag="trs")
                    nc.vector.tensor_copy(trs[0:ntiles, :],
                                          trp[0:ntiles, :])
                    return trs

                def load_group(g):
                    c0 = g * GRP
                    nc.sync.dma_start(
                        RF[:, :, c0:c0 + GRP],
                        AP(xt8_d, c0, [[N, 128], [128 * N, 2], [1, GRP]]))

                def load_oh(g):
                    gs = slice(g * GRP, (g + 1) * GRP)
                    nc.sync.dma_start(
                        OHJ[:, :, gs],
                        AP(ohj_d, g * GRP, [[N, 128], [128 * N, 2],
                                            [1, GRP]]))

                # ---- bulk loads first: the SP DMA queue must never
                # stall behind a dependency-gated transfer; each dma has a
                # ~625ns fixed cost so order = need order ----

                nc.sync.dma_start(
                    XO, AP(xto_d, 0, [[OWN, 128], [128 * OWN, 2],
                                      [1, OWN]]))
                nc.sync.dma_start(CB, cb_d[:])
                dumt = work.tile([128, 1], f32, tag="dum")
                nc.scalar.activation(dumt, onesb_sb, Act.Exp)
                nc.vector.tensor_mul(SO, XO, XO)
                invpo = work.tile([128, 16], bf16, tag="invpo")
                sumsq_lnexp(SO[:, 0, :], SO[:, 1, :], MT, invpo[:, 0:MT])

                # ---- global norm chain, per group ----

                def sq_stage(g, eng):
                    """Squares of group g's fp8 columns (SBUF->SBUF;
                    Pool-legal).  Emitted ~2 group-windows before use so
                    the slow Pool multiply never blocks a queue."""
                    gs = slice(g * GRP, (g + 1) * GRP)
                    sq2 = work.tile([128, 2, GRP], bf16, tag="sq2")
                    eng.tensor_mul(sq2, RF[:, :, gs], RF[:, :, gs])
                    return sq2

                def fin_stage(g, sq2):
                    """sumsq matmuls + packed ln/exp + ACT scale vectors
                    SCA (table exp, 10*inv) / SCP (Schraudolph, SA*inv).
                    Emitted one group-window before use."""
                    invp = work.tile([128, 16], f32, tag="invp")
                    sumsq_lnexp(sq2[:, 0, :], sq2[:, 1, :], GT,
                                invp[:, 0:GT])
                    sca = work.tile([128, GT], f32, tag="sca")
                    scp = work.tile([128, GT], f32, tag="scp")
                    nc.vector.tensor_scalar_mul(sca, invp[:, 0:GT], ISCALE)
                    nc.vector.tensor_scalar_mul(scp, invp[:, 0:GT], SA)
                    return sca, scp

                nc.sync.dma_start(sels_sb, sels_d[:])
                nc.sync.dma_start(CF, cf_d[:])
                load_group(0)
                load_group(1)
                load_group(2)
                load_group(3)
                load_oh(0)
                nc.sync.dma_start(
                    OHM, AP(ohm_d, 0, [[OWN, 128], [128 * OWN, 2],
                                       [1, OWN]]))
                load_oh(1)
                # own-row unpack + mults (gates the first main matmul);
                # bc tiles borrow the still-idle mm tag so the two chunk
                # chains overlap instead of ping-ponging on the ns tag
                trso = unpack_inv(invpo[:, 0:MT], MT)
                for c in range(OWN // CHUNK):
                    cs = slice(c * CHUNK, (c + 1) * CHUNK)
                    bc = psum.tile([128, OWN], f32, tag="mm")
                    for i in range(4):
                        t = c * 4 + i
                        nc.tensor.matmul(
                            bc[:, i * 128:(i + 1) * 128],
                            sels_sb[0:MT, t * 128:(t + 1) * 128],
                            trso[0:MT, :], start=True, stop=True)
                    nc.vector.tensor_mul(RFO[:, 0, cs], XO[:, 0, cs],
                                         bc[:, 0:512])
                    nc.vector.tensor_mul(RFO[:, 1, cs], XO[:, 1, cs],
                                         bc[:, 0:512])
                sq_g = {}
                sq_g[0] = sq_stage(0, nc.gpsimd)
                sq_g[1] = sq_stage(1, nc.vector)
                sc0 = fin_stage(0, sq_g.pop(0))

                # ---- main loop: 64 j-tiles, transposed orientation ----
                DENPS = dpsum.tile([128, MT], f32, tag="den")
                pending = []

                def flush_sums(limit):
                    # one psum accumulation group spans the whole DENPS
                    # bank: exactly one start and one stop
                    while len(pending) > limit:
                        t, ext, isf32 = pending.pop(0)
                        for s in range(MT):
                            sl = ext[:, s * 128:(s + 1) * 128]
                            st = (t == 0 and s == 0)
                            sp = (t == NT - 1 and s == MT - 1)
                            if isf32:
                                nc.tensor.matmul(
                                    DENPS[:, s:s + 1], sl.bitcast(f32),
                                    onesf_sb, start=st, stop=sp)
                            else:
                                nc.tensor.matmul(
                                    DENPS[:, s:s + 1], sl, onesb_sb,
                                    start=st, stop=sp)

                def main_tile(t, dve, sc):
                    # dve: "A" = ACT table exp, "D"/"P" = Schraudolph
                    sca, scp = sc
                    tl = t % GT
                    tb = slice(t * 128, (t + 1) * 128)
                    ps = psum.tile([128, OWN], f32, tag="mm")
                    for c in range(OWN // CHUNK):
                        cs = slice(c * CHUNK, (c + 1) * CHUNK)
                        nc.tensor.matmul(ps[:, cs], RF[:, :, tb],
                                         RFO[:, :, cs],
                                         start=True, stop=False,
                                         perf_mode=DR)
                    for c in range(OWN // CHUNK):
                        cs = slice(c * CHUNK, (c + 1) * CHUNK)
                        nc.tensor.matmul(ps[:, cs], OHJ[:, :, tb],
                                         OHM[:, :, cs],
                                         start=False, stop=True,
                                         perf_mode=DR)
                    if dve == "D":
                        # Schraudolph fast exp on DVE (Pool cannot read
                        # PSUM, so the split is ACT/DVE only)
                        ext = expool.tile([128, OWN], mybir.dt.int32,
                                          tag="exi")
                        nc.vector.tensor_scalar(
                            out=ext, in0=ps, scalar1=scp[:, tl:tl + 1],
                            scalar2=SB, op0=Alu.mult, op1=Alu.add)
                        pending.append((t, ext, True))
                    else:
                        ext = expool.tile([128, OWN], bf16, tag="ext")
                        nc.scalar.activation(ext, ps, Act.Exp,
                                             scale=sca[:, tl:tl + 1])
                        pending.append((t, ext, False))
                    flush_sums(LAG)


                sc = sc0
                sc_next = None
                for g in range(NG):
                    plan = PLAN_G[g]
                    for m in range(GT):
                        main_tile(g * GT + m, plan[m], sc)
                        if m == 1:
                            if g + 4 < NG:
                                load_group(g + 4)
                            if g + 2 < NG:
                                load_oh(g + 2)
                        if m == 6 and g == 0:
                            # CNT is only read in the finale; queue it
                            # after all startup-critical transfers
                            nc.sync.dma_start(CNT, cnt_d[:])
                        if m == 3 and g + 2 < NG:
                            sq_g[g + 2] = sq_stage(g + 2, nc.gpsimd)
                        if m == 6 and g + 1 < NG:
                            sc_next = fin_stage(g + 1, sq_g.pop(g + 1))
                    sc = sc_next
                flush_sums(0)

                # ---- finale: den = colsum + count + 1 -> log ----
                # stt reads DENPS straight from psum; ln fuses the row
                # reduce via its accumulator
                nc.vector.scalar_tensor_tensor(
                    out=DEN, in0=DENPS, scalar=1.0, in1=CNT,
                    op0=mybir.AluOpType.add, op1=mybir.AluOpType.add)
                nc.scalar.activation(LV, DEN, Act.Ln, accum_out=LS)
                psf = psum.tile([1, 1], f32, tag="mm")
                nc.tensor.matmul(psf, LS, onesf_sb, start=True, stop=True)
                nc.vector.tensor_copy(outsb, psf)
                nc.sync.dma_start(out_d[:], outsb)

        nc.compile()
    finally:
        bacc_mod.get_activation_tables = orig_gat
    return nc


def _get_nc():
    if "nc" not in _CACHE:
        _CACHE["nc"] = _build()
    return _CACHE["nc"]


def _make_in_maps(representations, pseudo_labels):
    x = np.asarray(representations, dtype=np.float32)
    labels = np.asarray(pseudo_labels).astype(np.int32).reshape(N)
    xt = np.ascontiguousarray(x.T).astype(ml_dtypes.bfloat16)   # [256, N]
    xt8 = xt.astype(ml_dtypes.float8_e4m3)                      # [256, N]

    oh = (labels[None, :] == np.arange(128, dtype=np.int32)[:, None])
    # one-hot channels with a zeroed twin slab (fp8 DoubleRow operand)
    ohj = np.zeros((256, N), dtype=ml_dtypes.float8_e4m3)
    ohj[0:128] = oh
    counts = np.bincount(labels, minlength=128).astype(np.float32)
    cnt_row = counts[labels]                                    # [N]

    in_maps = []
    for c in range(NCORES):
        lo, hi = c * OWN, (c + 1) * OWN
        ohm = np.zeros((256, OWN), dtype=ml_dtypes.float8_e4m3)
        ohm[0:128] = -80.0 * oh[:, lo:hi]
        cnt = np.ascontiguousarray(
            cnt_row[lo:hi].reshape(MT, 128).T).astype(np.float32)
        in_maps.append({
            "xt8": xt8,
            "xto": np.ascontiguousarray(xt[:, lo:hi]),
            "ohj": ohj,
            "ohm": ohm,
            "cnt": cnt,
        })
    return in_maps


def kernel(representations, pseudo_labels):
    from concourse.bass_utils import run_bass_kernel_spmd

    nc = _get_nc()
    in_maps = _make_in_maps(representations, pseudo_labels)
    res = run_bass_kernel_spmd(nc, in_maps, list(range(NCORES)))
    total = np.sum([np.float64(res.results[c]["out"][0, 0])
                    for c in range(NCORES)])
    return np.float32(total / N)
